# revision 1
# baseline (speedup 1.0000x reference)
"""Causal self-attention on 8 Trainium2 NeuronCores.

Sharding: core c = (batch b = c//2) x (head-half h2 = c%2). Each core
computes, for its batch and its 8 heads (of 16): the QKV projection
(only its W_qkv columns), causal flash attention, and a *partial*
output projection against its 512 rows of W_out. The host sums the
two half partials per batch and adds b_out. No device collectives.

v2 layout (all matmul inputs bf16 — same PE rate as f32r but half the
DMA/SBUF and no <256-wide fp32r penalty; PSUM accumulate stays fp32):

  xt[i]   (128c, 2048t) bf16   x[b]^T channel chunks, full T, loaded once
  wqkT/wvT/wo                  weights resident in SBUF, loaded once
  qk[j]   (128, T) bf16        Q^T pairs j=0..3, K^T pairs j=4..7
  vws[m]  (128t, 8*(64+1))     V' with fused ones-column (denominator)
  S^T     PSUM (k-part, q-free); exp on ACT (scale=1/8, causal mask via
          identity-matmul add on diagonal blocks); P in bf16
  O'^T    PSUM (65, q) accumulated over k-blocks; row 64 = denominator
  ot[j]   (128, T) bf16        normalized O^T pairs -> out-proj lhsT

Schedule (single PE stream, ACT-latency hidden by fillers):
  seg0: QKV for t<1024
  segA: attention q-group 0 per head, QKV-for-t>=1024 units interleaved
  segB: attention q-group 1 per head, out-proj chunks m<8 interleaved
  segC: out-proj chunks m>=8
PSUM: proj/out 2 banks + S 2x2 banks + O accum 2 banks = 8.
"""
import os
import sys

sys.path.insert(0, "/opt/trn_rl_repo")

import numpy as np

import concourse.bacc as bacc
import concourse.mybir as mybir
import concourse.tile as tile
from concourse.bass_utils import run_bass_kernel_spmd

B, T, C = 4, 2048, 1024
H = 16
HD = C // H              # 64
N_CORES = 8
HL = H // 2              # 8 local heads per core
CL = HL * HD             # 512 local channels
F32 = mybir.dt.float32
BF16 = mybir.dt.bfloat16

QG = 1024                # q-group width
KB = 128                 # k-block
TCH = 128                # t-chunk (tokens per out-proj M)
NTCH = T // TCH          # 16
CCH = 128                # channel chunk (contraction tile)
NCCH = C // CCH          # 8

_cache = {}


def _build(dbg=False, reps=1):
    nc = bacc.Bacc("TRN2", target_bir_lowering=False, debug=False,
                   num_devices=N_CORES)

    xT = nc.dram_tensor("xT", [C, T], BF16, kind="ExternalInput")
    wqk = nc.dram_tensor("wqk", [C, 2 * CL], BF16, kind="ExternalInput")
    wv = nc.dram_tensor("wv", [C, CL], BF16, kind="ExternalInput")
    wout = nc.dram_tensor("wout", [CL, C], BF16, kind="ExternalInput")
    mask = nc.dram_tensor("mask", [KB, KB], BF16, kind="ExternalInput")
    ident = nc.dram_tensor("ident", [KB, KB], BF16, kind="ExternalInput")
    y = nc.dram_tensor("y", [T, C], BF16, kind="ExternalOutput")

    with tile.TileContext(nc) as tc:
      for _rep in range(reps):
        with (
            tc.tile_pool(name="persist", bufs=1) as pp,
            tc.tile_pool(name="ps1", bufs=2, space="PSUM") as ps1,
            tc.tile_pool(name="pss", bufs=2, space="PSUM") as pss,
            tc.tile_pool(name="pso", bufs=1, space="PSUM") as pso,
            tc.tile_pool(name="p2", bufs=18) as p2,
            tc.tile_pool(name="p2n", bufs=2) as p2n,
            tc.tile_pool(name="p3y", bufs=3) as p3y,
        ):
            # ---- persistent SBUF tiles ----
            qk = [pp.tile([128, T], BF16, tag=f"qk{j}", name=f"qk{j}")
                  for j in range(8)]
            vws = [pp.tile([128, HL * (HD + 1)], BF16, tag=f"vw{m}",
                           name=f"vw{m}") for m in range(NTCH)]
            ot = [pp.tile([128, T], BF16, tag=f"ot{j}", name=f"ot{j}")
                  for j in range(4)]
            wo = [pp.tile([128, C], BF16, tag=f"wo{j}", name=f"wo{j}")
                  for j in range(4)]
            xt = [pp.tile([128, T], BF16, tag=f"xt{i}", name=f"xt{i}")
                  for i in range(NCCH)]
            wqkT = [pp.tile([128, 2 * CL], BF16, tag=f"wq{i}", name=f"wq{i}")
                    for i in range(NCCH)]
            wvT = [pp.tile([128, CL], BF16, tag=f"wv{i}", name=f"wv{i}")
                   for i in range(NCCH)]
            msk = pp.tile([KB, KB], BF16, tag="msk", name="msk")
            idn = pp.tile([KB, KB], BF16, tag="idn", name="idn")

            # ---- input DMAs (ordered so early compute can start: the
            # first units are V m-chunks, which need only wvT + early x
            # columns) ----
            for i in range(NCCH):
                nc.sync.dma_start(
                    wvT[i][:], wv[i * CCH:(i + 1) * CCH, :])
            for i in range(NCCH):
                nc.sync.dma_start(
                    xt[i][:, 0:T // 2], xT[i * CCH:(i + 1) * CCH, 0:T // 2])
            nc.sync.dma_start(msk[:], mask[:])
            nc.sync.dma_start(idn[:], ident[:])
            for i in range(NCCH):
                nc.sync.dma_start(
                    wqkT[i][:], wqk[i * CCH:(i + 1) * CCH, :])
            for i in range(NCCH):
                nc.sync.dma_start(
                    xt[i][:, T // 2:T], xT[i * CCH:(i + 1) * CCH, T // 2:T])
            for j in range(4):
                nc.sync.dma_start(wo[j][:], wout[j * 128:(j + 1) * 128, :])

            # ---- unit emitters ----
            def qk_unit(j, tg):
                # Q^T/K^T 128-row chunk j, 512-wide t-group tg
                ps = ps1.tile([128, 512], F32, tag="p1", name="mm")
                for i in range(NCCH):
                    nc.tensor.matmul(
                        ps[:],
                        wqkT[i][:, j * 128:(j + 1) * 128],
                        xt[i][:, tg * 512:(tg + 1) * 512],
                        start=(i == 0), stop=(i == NCCH - 1))
                dst = qk[j][:, tg * 512:(tg + 1) * 512]
                if (j + tg) % 2:
                    nc.vector.tensor_copy(dst, ps[:])
                else:
                    nc.scalar.copy(dst, ps[:])

            def v_unit(m):
                ps = ps1.tile([128, CL], F32, tag="p1", name="mmv")
                for i in range(NCCH):
                    nc.tensor.matmul(
                        ps[:],
                        xt[i][:, m * TCH:(m + 1) * TCH],
                        wvT[i][:],
                        start=(i == 0), stop=(i == NCCH - 1))
                vt = vws[m]
                dst = vt[:].rearrange("p (h x) -> p h x", x=HD + 1)
                nc.vector.tensor_copy(
                    dst[:, :, 0:HD],
                    ps[:].rearrange("p (h d) -> p h d", d=HD))
                nc.vector.memset(dst[:, :, HD:HD + 1], 1.0)

            def s_block(h, qlo, kb):
                # S^T block: lhsT = K^T slice, rhs = Q^T; causal mask on the
                # diagonal accumulated via identity-weighted matmul so the
                # PSUM->exp chain stays PE->ACT. Returns bf16 P tile.
                jp, pb = h // 2, (h % 2) * 64
                r0 = max(0, kb * KB - qlo)
                s_ps = pss.tile([128, QG], F32, tag="sps", name="sps")
                diag = kb * KB >= qlo
                lhs = qk[4 + jp][pb:pb + 64, kb * KB:(kb + 1) * KB]
                c0 = r0
                while c0 < QG:
                    c1 = min(QG, (c0 // 512 + 1) * 512)
                    nc.tensor.matmul(
                        s_ps[:, c0:c1],
                        lhs,
                        qk[jp][pb:pb + 64, qlo + c0:qlo + c1],
                        start=True,
                        stop=(not diag) or (c0 != r0))
                    c0 = c1
                if diag:
                    nc.tensor.matmul(
                        s_ps[:, r0:r0 + KB], idn[:], msk[:],
                        start=False, stop=True)
                p_sb = p2.tile([128, QG], BF16, tag="p", name="p")
                nc.scalar.activation(
                    p_sb[:, r0:], s_ps[:, r0:],
                    mybir.ActivationFunctionType.Exp,
                    scale=0.125)
                return p_sb

            def pv_block(h, qlo, nkb, kb, p_sb, o_ps):
                # P@V' accumulate: rows 0..63 = O^T, row 64 = denominator
                r0 = max(0, kb * KB - qlo)
                lhv = vws[kb][:, h * (HD + 1):(h + 1) * (HD + 1)]
                c0 = (r0 // 512) * 512
                while c0 < QG:
                    c1 = min(QG, c0 + 512)
                    rs = max(c0, r0)
                    last_kb = min(nkb, (qlo + c1) // KB) - 1
                    nc.tensor.matmul(
                        o_ps[0:HD + 1, rs:c1],
                        lhv,
                        p_sb[:, rs:c1],
                        start=(kb == 0), stop=(kb == last_kb))
                    c0 = c1

            def normalize(h, qlo, o_ps):
                # recip of denom row (lane-locked on partition 64), DMA-hop
                # to partition 0, gpsimd broadcast, DVE mul -> ot (bf16).
                jp, pb = h // 2, (h % 2) * 64
                rr = p2n.tile([65, QG], F32, tag="rr", name="rr")
                nc.vector.reciprocal(rr[64:65, :], o_ps[HD:HD + 1, :])
                rr0 = p2n.tile([1, QG], F32, tag="rr0", name="rr0")
                nc.sync.dma_start(rr0[:], rr[64:65, :])
                rb = p2n.tile([64, QG], F32, tag="rb", name="rb")
                nc.gpsimd.partition_broadcast(rb[:], rr0[:])
                if pb == 0:
                    nc.vector.tensor_mul(
                        ot[jp][0:64, qlo:qlo + QG], o_ps[0:HD, :], rb[:])
                else:
                    os_ = p2n.tile([64, QG], BF16, tag="os", name="os")
                    nc.vector.tensor_mul(os_[:], o_ps[0:HD, :], rb[:])
                    nc.sync.dma_start(
                        ot[jp][64:128, qlo:qlo + QG], os_[:])

            def p3_unit(m, n):
                ps = ps1.tile([128, 512], F32, tag="p1", name="mm3")
                for j in range(4):
                    nc.tensor.matmul(
                        ps[:],
                        ot[j][:, m * TCH:(m + 1) * TCH],
                        wo[j][:, n * 512:(n + 1) * 512],
                        start=(j == 0), stop=(j == 3))
                ysb = p3y.tile([128, 512], BF16, tag="y", name="y")
                nc.vector.tensor_copy(ysb[:], ps[:])
                nc.sync.dma_start(
                    y[m * TCH:(m + 1) * TCH, n * 512:(n + 1) * 512],
                    ysb[:])

            # ---- seg0: QKV for t<1024 (V first: it only needs wvT +
            # early x columns, so compute starts before wqkT lands) ----
            for m in range(8):
                v_unit(m)
            for j in (4, 5, 6, 7, 0, 1, 2, 3):
                qk_unit(j, 0)
                qk_unit(j, 1)

            # ---- segA: q-group 0 + QKV-t>=1024 fillers ----
            fillers = [(qk_unit, (j, tg))
                       for j in (4, 5, 6, 7, 0, 1, 2, 3) for tg in (2, 3)]
            fillers += [(v_unit, (m,)) for m in range(8, 16)]
            fi = 0
            for h in range(HL):
                nkb = QG // KB
                o_ps = pso.tile([128, QG], F32, tag="ops", name="ops")
                pbs = [s_block(h, 0, kb) for kb in range(nkb)]
                for _ in range(3):
                    if fi < len(fillers):
                        f, a = fillers[fi]
                        f(*a)
                        fi += 1
                for kb in range(nkb):
                    pv_block(h, 0, nkb, kb, pbs[kb], o_ps)
                normalize(h, 0, o_ps)
            while fi < len(fillers):
                f, a = fillers[fi]
                f(*a)
                fi += 1

            # ---- segB: q-group 1 + out-proj m<8 fillers ----
            # odd heads (pb=64, extra SBUF-SBUF DMA in normalize) first so
            # the final normalize before segC is a direct DVE write
            for hi, h in enumerate((1, 3, 5, 7, 0, 2, 4, 6)):
                nkb = 2 * QG // KB
                o_ps = pso.tile([128, QG], F32, tag="ops", name="ops")
                pbs = [s_block(h, QG, kb) for kb in range(nkb)]
                for kb in range(nkb):
                    pv_block(h, QG, nkb, kb, pbs[kb], o_ps)
                normalize(h, QG, o_ps)
                p3_unit(hi, 0)
                p3_unit(hi, 1)

            # ---- segC: out-proj m>=8 ----
            for m in range(8, NTCH):
                for n in range(2):
                    p3_unit(m, n)

    nc.compile()
    return nc


def make_in_maps(x, W_qkv, W_out):
    import ml_dtypes
    bf = ml_dtypes.bfloat16
    mask = np.where(
        np.arange(KB)[None, :] < np.arange(KB)[:, None], -1e30, 0.0
    ).astype(bf)
    ident = np.eye(KB).astype(bf)

    in_maps = []
    for c in range(N_CORES):
        b, h2 = c // 2, c % 2
        cols = slice(h2 * CL, (h2 + 1) * CL)
        in_maps.append({
            "xT": np.ascontiguousarray(x[b].T).astype(bf),
            "wqk": np.ascontiguousarray(
                np.concatenate([W_qkv[:, cols],
                                W_qkv[:, C:][:, cols]], axis=1)).astype(bf),
            "wv": np.ascontiguousarray(W_qkv[:, 2 * C:][:, cols]).astype(bf),
            "wout": np.ascontiguousarray(W_out[cols, :]).astype(bf),
            "mask": mask,
            "ident": ident,
        })
    return in_maps


def kernel(x, W_qkv, b_qkv, W_out, b_out, _trace=False):
    x = np.asarray(x, dtype=np.float32)
    W_qkv = np.asarray(W_qkv, dtype=np.float32)
    b_qkv = np.asarray(b_qkv, dtype=np.float32)
    W_out = np.asarray(W_out, dtype=np.float32)
    b_out = np.asarray(b_out, dtype=np.float32)

    # q/k biases would need device-side adds; this problem pins them to 0.
    assert not b_qkv[:2 * C].any(), "nonzero q/k bias unsupported"

    if "nc" not in _cache:
        _cache["nc"] = _build()
    nc = _cache["nc"]

    in_maps = make_in_maps(x, W_qkv, W_out)

    kwargs = {}
    if _trace:
        kwargs = {"trace": True, "trace_cores": [0]}
    res = run_bass_kernel_spmd(nc, in_maps, core_ids=list(range(N_CORES)),
                               **kwargs)

    out = np.empty((B, T, C), dtype=np.float32)
    # v-bias passes through softmax as +b_v, so it folds into the output
    # projection; b_out likewise. Both are host-side adds on the partials.
    bias = b_qkv[2 * C:] @ W_out + b_out
    for b in range(B):
        out[b] = (res.results[2 * b]["y"].astype(np.float32)
                  + res.results[2 * b + 1]["y"].astype(np.float32) + bias)
    if _trace:
        kernel.last_exec_ns = res.exec_time_ns
        kernel.last_trace = (res.instructions_and_trace or (None, None))[1]
    return out

